# revision 27
# baseline (speedup 1.0000x reference)
"""Multi-head attention (B=4, T=2048, E=2048, H=16) on 8 trn2 NeuronCores.

Sharding: batch x head-half. Core c handles batch b = c//2 and heads
half*8..half*8+8 where half = c%2 (Megatron-style: Wq/Wk/Wv row-split,
Wo column-split; the two partial outputs per batch are summed on host,
where the output bias is also added).

Per-core device pipeline (fp32 PSUM accumulate everywhere), ordered
Q-proj -> V-proj -> K-proj -> attention -> out-proj so that every
phase's staging DMA prefetches inside the previous phase's compute
window without exceeding the ~208KB/partition SBUF cap:
  1. projections   Q^T = Wq_c @ x_q^T and K^T = Wk_c @ x_k^T in fp8
                   DoubleRow (inputs+weights quantized to e4m3 on host,
                   weights pre-scaled by 2^6 into fp8's normal range,
                   compensated for free inside the exp scale); V fp16.
  2. attention     flattened (block, pair) software pipeline: S^T tile
                   = K_h Q_h^T (fp16, contract d=128), exp on ACT (no
                   max-subtraction: |S*scale| <= ~2.5 here), softmax
                   denominators via an fp8 DoubleRow all-ones matmul
                   over an fp8 copy of exp(S^T) (the 2048-term sum
                   averages the quantization noise away) — emitted two
                   slots late so its DVE-cast input is always ready no
                   matter where the walrus scheduler places it within
                   the slot, O^T = V_h^T @ exp(S^T) in fp16, normalize
                   with reciprocal_approx_fast + tensor_mul on DVE.
                   The S pair for slot s+1 is emitted under
                   tc.high_priority so the scheduler pins it ahead of
                   the exp-gated O matmuls; with that, the phase runs
                   PE-bound at ~1091ns/slot (5 matmuls x 216ns + sem),
                   with the exp stream (853ns compute + ~260ns fixed
                   access latency per instruction) just underneath.
                   The PSUM budget (2x st double-buffered + ot + sm
                   accumulators = 8 banks exactly) blocks wider exp
                   tiles.
  3. out-proj      P = O @ Wo_c^T in fp16 (weights preloaded during
                   attention; host adds the core-pair + bias).

The tensor engine streams 1 moving column/cycle regardless of dtype;
fp8 DoubleRow's 2x comes from 256-deep contraction per matmul, so the
2816 matmuls' 1.44M columns set a ~610us floor that every phase runs
within a few percent of.

Error budget (measured, vs f64 reference, gate 2e-2): 16-bit baseline
3.6e-3; +fp8 Q/K path ~1.84e-2. fp8 for V/out-proj measures 3.5e-2+
and is rejected: any random-sign contraction with elementwise fp8
quantization costs ~3.6% of output std; only the Q/K logit path
survives because the 1/sqrt(d) logit scale damps it through softmax.

DMA: one trigger per tensor chunk (sync engine costs ~0.6us/trigger);
weights resident per phase; x streamed in 1MB token blocks; the first
wq chunk is split so the boot-critical first matmul depends on ~32KB;
the final output group is DMA'd per token-tile to shorten the drain.
"""
import os
import sys
import math
from contextlib import ExitStack

if os.path.isdir("/opt/trn_rl_repo") and "/opt/trn_rl_repo" not in sys.path:
    sys.path.insert(0, "/opt/trn_rl_repo")

import numpy as np
import ml_dtypes

import concourse.bass as bass
import concourse.tile as tile
from concourse import bacc, mybir
from concourse.bass_utils import run_bass_kernel_spmd

EMBED, HEADS, B, T = 2048, 16, 4, 2048
HD = EMBED // HEADS          # 128 head dim
NCORES = 8
HPC = HEADS // 2             # 8 heads per core
CD = HPC * HD                # 1024 local head-concat dim
SCALE = 1.0 / math.sqrt(HD)
# Wq/Wk are pre-scaled by 2**6 on the host so their ~U(-0.022, 0.022)
# entries land in fp8e4's normal range (min normal 2^-6) instead of being
# crushed to subnormals. Q'K'^T = 4096 * QK^T; fold the compensation into
# the exp scale for free.
WSCALE = 64.0
SCALE_Q = SCALE / (WSCALE * WSCALE)

F32 = mybir.dt.float32
F16 = mybir.dt.float16
FP8 = mybir.dt.float8e4
F16_NP = np.float16
F8_NP = ml_dtypes.float8_e4m3

_CACHE = {}


def _build():
    nc = bacc.Bacc("TRN2", target_bir_lowering=False, debug=False,
                   num_devices=NCORES)
    xq = nc.dram_tensor("xq", [EMBED, T], FP8, kind="ExternalInput").ap()
    xk = nc.dram_tensor("xk", [EMBED, T], FP8, kind="ExternalInput").ap()
    xv = nc.dram_tensor("xv", [EMBED, T], F16, kind="ExternalInput").ap()
    wq = nc.dram_tensor("wq", [EMBED, CD], FP8, kind="ExternalInput").ap()
    wk = nc.dram_tensor("wk", [EMBED, CD], FP8, kind="ExternalInput").ap()
    wv = nc.dram_tensor("wv", [EMBED, CD], F16, kind="ExternalInput").ap()
    wo = nc.dram_tensor("wo", [CD, EMBED], F16, kind="ExternalInput").ap()
    p = nc.dram_tensor("p", [T, EMBED], F32, kind="ExternalOutput").ap()

    ET = EMBED // 128        # 16 contraction tiles over embed
    XB = 512                 # token width of streamed x blocks
    NTB = T // XB            # 4
    DR = mybir.MatmulPerfMode.DoubleRow

    with tile.TileContext(nc) as tc, ExitStack() as ctx:
        with ExitStack() as qkv_ctx:
            qt_pool = qkv_ctx.enter_context(tc.tile_pool(name="qt", bufs=1))
            kt_pool = qkv_ctx.enter_context(tc.tile_pool(name="kt", bufs=1))
            v_pool = qkv_ctx.enter_context(tc.tile_pool(name="v", bufs=1))
            qt_sb = qt_pool.tile([128, HPC, T], F16)  # Q^T: [d, h, q]
            kt_sb = kt_pool.tile([128, HPC, T], F16)  # K^T: [d, h, k]
            v_sb = v_pool.tile([128, T // 128, CD], F16)  # V: [tok, tt, c]

            xq_r = xq.rearrange("(e p) t -> p e t", p=128)
            xk_r = xk.rearrange("(e p) t -> p e t", p=128)
            xv_r = xv.rearrange("(e p) t -> p e t", p=128)
            wq_r = wq.rearrange("(e p) c -> p e c", p=128)
            wk_r = wk.rearrange("(e p) c -> p e c", p=128)
            wv_r = wv.rearrange("(e p) c -> p e c", p=128)
            EH = ET // 2

            def proj_qk(x_r, w_sb, out_sb, load_xb):
                # one [128,512]-psum group per (tb, ds): contract embed via
                # 8 fp8 DoubleRow matmuls
                for tb in range(NTB):
                    xb = load_xb(tb)
                    for ds in range(HPC):
                        pst = ps1.tile([128, XB], F32, tag="pp", name="pst")
                        for e in range(0, ET, 2):
                            nc.tensor.matmul(
                                pst[:],
                                w_sb[:, e:e + 2, ds * 128:(ds + 1) * 128],
                                xb[:, e:e + 2, :],
                                start=(e == 0), stop=(e == ET - 2),
                                perf_mode=DR)
                        nc.vector.tensor_copy(
                            out_sb[:, ds, tb * XB:(tb + 1) * XB], pst[:])

            with ExitStack() as pv:
                # V staging pools (outlive the Q and K scopes below)
                wvpool = pv.enter_context(tc.tile_pool(name="w1", bufs=1))
                xvpool = pv.enter_context(tc.tile_pool(name="x1", bufs=2))

                # ---------------- phase 1a: Q projection (fp8 DR) -------
                with ExitStack() as p1:
                    w8pool = p1.enter_context(tc.tile_pool(name="w18q",
                                                           bufs=1))
                    x8pool = p1.enter_context(tc.tile_pool(name="x18q",
                                                           bufs=3))
                    ps1 = p1.enter_context(
                        tc.tile_pool(name="ps1", bufs=4, space="PSUM"))

                    wq_sb = w8pool.tile([128, ET, CD], FP8, tag="w8")
                    xqb = [None] * NTB

                    def load_xq(tb):
                        xb = x8pool.tile([128, ET, XB], FP8, tag="xb8",
                                         name="xb")
                        nc.sync.dma_start(
                            out=xb[:, :, :],
                            in_=xq_r[:, :, tb * XB:(tb + 1) * XB])
                        return xb

                    # q-critical loads first in fine-grained interleaved
                    # chunks, then v staging prefetch.  The first psum
                    # group only touches w[:, 0:2, 0:128] and x[:, 0:2, :];
                    # splitting those descriptors out makes the first
                    # LDWEIGHTS depend on ~32KB.
                    xqb[0] = x8pool.tile([128, ET, XB], FP8, tag="xb8",
                                         name="xb")
                    EQ = ET // 8
                    nc.scalar.dma_start(out=xqb[0][:, 0:EQ, :],
                                        in_=xq_r[:, 0:EQ, 0:XB])
                    nc.sync.dma_start(out=wq_sb[:, 0:EQ, 0:128],
                                      in_=wq_r[:, 0:EQ, 0:128])
                    nc.sync.dma_start(out=wq_sb[:, 0:EQ, 128:CD],
                                      in_=wq_r[:, 0:EQ, 128:CD])
                    for c in range(1, 8):
                        es = slice(c * EQ, (c + 1) * EQ)
                        # issue the boot-critical loads from two otherwise
                        # idle engine queues in parallel with sync's
                        # preamble.  The first psum group contracts all of
                        # e, so it gates on xq block 0 (1MB, scalar queue)
                        # AND the full wq (2MB): each DMA queue sustains
                        # ~120GB/s at boot, so two wq chunks ride the
                        # scalar queue to balance the streams (~1.5MB each)
                        nc.scalar.dma_start(out=xqb[0][:, es, :],
                                            in_=xq_r[:, es, 0:XB])
                        weng = nc.scalar if c >= 6 else nc.sync
                        weng.dma_start(out=wq_sb[:, es, :],
                                       in_=wq_r[:, es, :])
                    xqb[1] = load_xq(1)
                    xqb[2] = load_xq(2)

                    wv_sb = wvpool.tile([128, ET, CD], F16, tag="w",
                                        name="wv_sb")
                    nc.sync.dma_start(out=wv_sb[:, 0:EH, :],
                                      in_=wv_r[:, 0:EH, :])
                    nc.sync.dma_start(out=wv_sb[:, EH:ET, :],
                                      in_=wv_r[:, EH:ET, :])

                    def load_xv(tb):
                        xb = xvpool.tile([128, ET, XB], F16, tag="xb",
                                         name="xvb")
                        nc.sync.dma_start(
                            out=xb[:, :, :],
                            in_=xv_r[:, :, tb * XB:(tb + 1) * XB])
                        return xb

                    xvb = load_xv(0)

                    def q_feed(tb):
                        if tb == NTB - 1:
                            xqb[3] = load_xq(3)
                        return xqb[tb]

                    proj_qk(xq_r, wq_sb, qt_sb, q_feed)

                # ------------- phase 1a': K staging + V projection ------
                with ExitStack() as p1:
                    w8pool = p1.enter_context(tc.tile_pool(name="w18k",
                                                           bufs=1))
                    x8pool = p1.enter_context(tc.tile_pool(name="x18k",
                                                           bufs=3))
                    ps1v = p1.enter_context(
                        tc.tile_pool(name="ps1v", bufs=4, space="PSUM"))

                    wk_sb = w8pool.tile([128, ET, CD], FP8, tag="w8")
                    xkb = [None] * NTB

                    def load_xk(tb):
                        xb = x8pool.tile([128, ET, XB], FP8, tag="xb8",
                                         name="xb")
                        nc.sync.dma_start(
                            out=xb[:, :, :],
                            in_=xk_r[:, :, tb * XB:(tb + 1) * XB])
                        return xb

                    nc.sync.dma_start(out=wk_sb[:, 0:EH, :],
                                      in_=wk_r[:, 0:EH, :])
                    xkb[0] = load_xk(0)

                    for tb in range(NTB):
                        xb = xvb if tb == 0 else load_xv(tb)
                        if tb == 1:
                            nc.sync.dma_start(out=wk_sb[:, EH:ET, :],
                                              in_=wk_r[:, EH:ET, :])
                            xkb[1] = load_xk(1)
                        for ts in range(XB // 128):
                            tt = tb * (XB // 128) + ts
                            for db in range(CD // 512):
                                pst = ps1v.tile([128, 512], F32, tag="ppv",
                                                name="pst")
                                for e in range(ET):
                                    nc.tensor.matmul(
                                        pst[:],
                                        xb[:, e, ts * 128:(ts + 1) * 128],
                                        wv_sb[:, e, db * 512:(db + 1) * 512],
                                        start=(e == 0), stop=(e == ET - 1))
                                nc.vector.tensor_copy(
                                    v_sb[:, tt, db * 512:(db + 1) * 512],
                                    pst[:])

                    # ------------- phase 1a'': K projection (fp8 DR) ----
                    ps1 = ps1v

                    def k_feed(tb):
                        if tb >= 2:
                            xkb[tb] = load_xk(tb)
                        return xkb[tb]

                    proj_qk(xk_r, wk_sb, kt_sb, k_feed)

            # o_pool created only now (lands in the staging space freed
            # above) to keep phase-1 SBUF under the per-partition cap
            o_pool = qkv_ctx.enter_context(tc.tile_pool(name="o", bufs=1))
            o_sb = o_pool.tile([128, HPC, T], F16)   # O^T: [d, h, q]

            # out-proj weights prefetch during attention (resident; the
            # phase-3 matmuls then never wait on DMA)
            wopool = qkv_ctx.enter_context(tc.tile_pool(name="wo3", bufs=1))
            wo_sb = wopool.tile([128, HPC, EMBED], F16, tag="wo")
            wo_r = wo.rearrange("(c p) e -> p c e", p=128)
            nc.sync.dma_start(out=wo_sb[:, :, 0:EMBED // 2],
                              in_=wo_r[:, :, 0:EMBED // 2])
            nc.sync.dma_start(out=wo_sb[:, :, EMBED // 2:EMBED],
                              in_=wo_r[:, :, EMBED // 2:EMBED])

            # ---------------- phase 2: attention ----------------
            with ExitStack() as p2:
                # deep e pools: exp(s) has a WAR wait on O(s-bufs) and
                # cast(s-bufs) via its pool slot; at 6 deep every PE
                # hiccup (block boundaries) propagated into the ACT
                # stream.  12 slots of lag absorb them.
                epool = p2.enter_context(tc.tile_pool(name="e2", bufs=14))
                e8pool = p2.enter_context(tc.tile_pool(name="e8", bufs=10))
                rpool = p2.enter_context(tc.tile_pool(name="r2", bufs=2))
                cpool = p2.enter_context(tc.tile_pool(name="c2", bufs=1))
                stps = p2.enter_context(
                    tc.tile_pool(name="st", bufs=2, space="PSUM"))
                otps = p2.enter_context(
                    tc.tile_pool(name="otp", bufs=2, space="PSUM"))
                smps = p2.enter_context(
                    tc.tile_pool(name="sm", bufs=2, space="PSUM"))

                # all-ones fp8 stationary [128, 2, 128]: one DoubleRow
                # matmul per exp-pair contracts 256 k-positions and lands the
                # softmax denominators replicated on every partition (full
                # width so the normalize multiply needs no cross-partition
                # broadcast)
                ones8 = cpool.tile([128, 2, 128], FP8)
                nc.vector.memset(ones8[:], 1.0)

                KT_N = T // 128   # 16 k tiles
                NP = KT_N // 2    # 8 pairs; exp runs on [128, 1024]

                def emit_st_pair(h, qb, pi):
                    # high_priority pins the S matmuls ahead of the
                    # exp-gated O matmuls of the previous slot in the
                    # scheduled PE order — otherwise the prefetched pair
                    # lands at the slot's end and the ACT stream sees it
                    # ~190ns late at every block boundary.
                    st = stps.tile([128, 1024], F32, tag="st", name="st")
                    with tc.high_priority(offset=64):
                        for j in range(2):
                            kt = 2 * pi + j
                            nc.tensor.matmul(
                                st[:, j * 512:(j + 1) * 512],
                                kt_sb[:, h, kt * 128:(kt + 1) * 128],
                                qt_sb[:, h, qb * 512:(qb + 1) * 512],
                                start=True, stop=True)
                    return st

                # Flattened (block, pair) pipeline. Slot s does:
                #   - st matmuls for slot s+1 (PE; its psum WAR is long
                #     resolved)
                #   - exp for slot s (ACT, self-paced back-to-back)
                #   - fp8 cast for slot s (DVE)
                #   - O matmuls for slot s (PE, gated only by exp s)
                #   - sm matmul for slot s-2 (PE). Two slots late because
                #     the walrus scheduler is free to hoist it ahead of the
                #     O matmuls within its slot, and its input (the DVE
                #     cast, itself gated on exp) only lands ~1.7 slots after
                #     the slot opens; at two slots' distance the PE never
                #     waits on it regardless of placement.
                blocks = [(h, qb) for h in range(HPC)
                          for qb in range(T // 512)]
                slots = [(bi, pi) for bi in range(len(blocks))
                         for pi in range(NP)]
                NS = len(slots)

                ots, sms = {}, {}
                e8s = {}

                def finish_block(b):
                    h, qb = blocks[b]
                    rbc = rpool.tile([128, 512], F32, tag="rbc",
                                     name="rbc")
                    nc.vector.reciprocal_approx_fast(rbc[:], sms[b][:])
                    nc.vector.tensor_mul(
                        o_sb[:, h, qb * 512:(qb + 1) * 512],
                        ots[b][:], rbc[:])
                    del ots[b], sms[b]

                def emit_sm(s):
                    b2, p2_ = slots[s]
                    nc.tensor.matmul(
                        sms[b2][:], ones8[:], e8s.pop(s)[:],
                        start=(p2_ == 0), stop=(p2_ == NP - 1),
                        perf_mode=DR)
                    if p2_ == NP - 1:
                        finish_block(b2)

                st_cur = emit_st_pair(*blocks[0], 0)
                st_nxt = None
                for si, (bi, pi) in enumerate(slots):
                    h, qb = blocks[bi]
                    if pi == 0:
                        ots[bi] = otps.tile([128, 512], F32, tag="ot",
                                            name="ot")
                        sms[bi] = smps.tile([128, 512], F32, tag="sm",
                                            name="sm")
                    # prefetch next slot's S pair so ACT never drains
                    if si + 1 < NS:
                        nbi, npi = slots[si + 1]
                        st_nxt = emit_st_pair(*blocks[nbi], npi)
                    e_sb = epool.tile([128, 1024], F16, tag="e",
                                      name="e_sb")
                    nc.scalar.activation(
                        e_sb[:], st_cur[:],
                        mybir.ActivationFunctionType.Exp,
                        scale=SCALE_Q)
                    e8s[si] = e8pool.tile([128, 2, 512], FP8, tag="e8",
                                          name="e8")
                    nc.vector.tensor_copy(e8s[si][:], e_sb[:])
                    for j in range(2):
                        kt = 2 * pi + j
                        nc.tensor.matmul(
                            ots[bi][:],
                            v_sb[:, kt, h * 128:(h + 1) * 128],
                            e_sb[:, j * 512:(j + 1) * 512],
                            start=(kt == 0), stop=(kt == KT_N - 1))
                    if si >= 2:
                        emit_sm(si - 2)
                    st_cur = st_nxt
                # drain the final two slots' denominators
                emit_sm(NS - 2)
                emit_sm(NS - 1)

            # ---------------- phase 3: output projection ----------------
            with ExitStack() as p3:
                ppool = p3.enter_context(tc.tile_pool(name="po3", bufs=4))
                ps3 = p3.enter_context(
                    tc.tile_pool(name="ps3", bufs=4, space="PSUM"))
                # batch output writes: 4 token tiles per DMA trigger (16
                # triggers instead of 64 on the in-order sync queue); the
                # final group goes out per-tile so the drain tail is a
                # 256KB transfer, not 1MB
                p_r = p.rearrange("(tt p) e -> p tt e", p=128)
                NEB, NTG = EMBED // 512, T // 512
                for eb in range(NEB):
                    for tg in range(NTG):
                        last = (eb == NEB - 1 and tg == NTG - 1)
                        pg = ppool.tile([128, 4, 512], F32, tag="po",
                                        name="pg")
                        for ti in range(4):
                            tt = tg * 4 + ti
                            pst = ps3.tile([128, 512], F32, tag="pp3",
                                           name="pst")
                            for ct in range(HPC):
                                nc.tensor.matmul(
                                    pst[:],
                                    o_sb[:, ct, tt * 128:(tt + 1) * 128],
                                    wo_sb[:, ct, eb * 512:(eb + 1) * 512],
                                    start=(ct == 0), stop=(ct == HPC - 1))
                            nc.scalar.copy(pg[:, ti, :], pst[:])
                            if last:
                                nc.sync.dma_start(
                                    out=p_r[:, tt:tt + 1,
                                            eb * 512:(eb + 1) * 512],
                                    in_=pg[:, ti:ti + 1, :])
                        if not last:
                            nc.sync.dma_start(
                                out=p_r[:, tg * 4:(tg + 1) * 4,
                                        eb * 512:(eb + 1) * 512],
                                in_=pg[:, :, :])

    nc.compile()
    return nc


def _get_nc():
    if "nc" not in _CACHE:
        _CACHE["nc"] = _build()
    return _CACHE["nc"]


def kernel(k, q, v, Wk, Wq, Wv, Wo, bo, _trace=False):
    k = np.asarray(k, dtype=np.float32)
    q = np.asarray(q, dtype=np.float32)
    v = np.asarray(v, dtype=np.float32)
    Wk = np.asarray(Wk, dtype=np.float32)
    Wq = np.asarray(Wq, dtype=np.float32)
    Wv = np.asarray(Wv, dtype=np.float32)
    Wo = np.asarray(Wo, dtype=np.float32)
    bo = np.asarray(bo, dtype=np.float32)

    nc = _get_nc()

    # host-side shard prep (q/k in fp8, v in fp16)
    xqT = [np.ascontiguousarray(q[b].T).astype(F8_NP) for b in range(B)]
    xkT = [np.ascontiguousarray(k[b].T).astype(F8_NP) for b in range(B)]
    xvT = [np.ascontiguousarray(v[b].T).astype(F16_NP) for b in range(B)]
    WqT = (Wq.T * WSCALE).astype(F8_NP)
    WkT = (Wk.T * WSCALE).astype(F8_NP)
    WvT = Wv.T.astype(F16_NP)
    WoT = Wo.T.astype(F16_NP)

    in_maps = []
    for c in range(NCORES):
        b, half = divmod(c, 2)
        sl = slice(half * CD, (half + 1) * CD)
        in_maps.append({
            "xq": xqT[b], "xk": xkT[b], "xv": xvT[b],
            "wq": np.ascontiguousarray(WqT[:, sl]),
            "wk": np.ascontiguousarray(WkT[:, sl]),
            "wv": np.ascontiguousarray(WvT[:, sl]),
            "wo": np.ascontiguousarray(WoT[sl, :]),
        })

    if _trace:
        try:
            res = run_bass_kernel_spmd(nc, in_maps, list(range(NCORES)),
                                       trace=True)
        except Exception as e:
            print(f"trace run failed ({e!r}); retrying without trace",
                  file=sys.stderr)
            res = run_bass_kernel_spmd(nc, in_maps, list(range(NCORES)))
    else:
        res = run_bass_kernel_spmd(nc, in_maps, list(range(NCORES)))
    _CACHE["exec_time_ns"] = res.exec_time_ns
    _CACHE["trace"] = res.instructions_and_trace

    out = np.empty((B, T, EMBED), dtype=np.float32)
    for b in range(B):
        out[b] = res.results[2 * b]["p"] + res.results[2 * b + 1]["p"] + bo
    return out


# revision 28
# speedup vs baseline: 1.1947x; 1.1947x over previous
"""Multi-head attention (B=4, T=2048, E=2048, H=16) on 8 trn2 NeuronCores.

Sharding: batch x head-half. Core c handles batch b = c//2 and heads
half*8..half*8+8 where half = c%2 (Megatron-style: Wq/Wk/Wv row-split,
Wo column-split; the two partial outputs per batch are summed on host,
where the output bias is also added).

Per-core device pipeline (fp32 PSUM accumulate everywhere), ordered
Q-proj -> V-proj -> K-proj -> attention -> out-proj so that every
phase's staging DMA prefetches inside the previous phase's compute
window without exceeding the ~208KB/partition SBUF cap:
  1. projections   Q^T = Wq_c @ x_q^T and K^T = Wk_c @ x_k^T in fp8
                   DoubleRow (inputs+weights quantized to e4m3 on host,
                   weights pre-scaled by 2^6 into fp8's normal range,
                   compensated for free inside the exp scale); V fp16.
  2. attention     flattened (block, pair) software pipeline: S^T tile
                   = K_h Q_h^T (fp16, contract d=128), exp on ACT (no
                   max-subtraction: |S*scale| <= ~2.5 here), softmax
                   denominators via an fp8 DoubleRow all-ones matmul
                   over an fp8 copy of exp(S^T) (the 2048-term sum
                   averages the quantization noise away) — emitted two
                   slots late so its DVE-cast input is always ready no
                   matter where the walrus scheduler places it within
                   the slot, O^T = V_h^T @ exp(S^T) in fp16, normalize
                   with reciprocal_approx_fast + tensor_mul on DVE.
                   The S pair for slot s+1 is emitted under
                   tc.high_priority so the scheduler pins it ahead of
                   the exp-gated O matmuls; with that, the phase runs
                   PE-bound at ~1091ns/slot (5 matmuls x 216ns + sem),
                   with the exp stream (853ns compute + ~260ns fixed
                   access latency per instruction) just underneath.
                   The PSUM budget (2x st double-buffered + ot + sm
                   accumulators = 8 banks exactly) blocks wider exp
                   tiles.
  3. out-proj      P = O @ Wo_c^T in fp16 (weights preloaded during
                   attention; host adds the core-pair + bias).

The tensor engine streams 1 moving column/cycle regardless of dtype;
fp8 DoubleRow's 2x comes from 256-deep contraction per matmul, so the
2816 matmuls' 1.44M columns set a ~610us floor that every phase runs
within a few percent of.

Error budget (measured, vs f64 reference, gate 2e-2): 16-bit baseline
3.6e-3; +fp8 Q/K path ~1.84e-2. fp8 for V/out-proj measures 3.5e-2+
and is rejected: any random-sign contraction with elementwise fp8
quantization costs ~3.6% of output std; only the Q/K logit path
survives because the 1/sqrt(d) logit scale damps it through softmax.

DMA: one trigger per tensor chunk (sync engine costs ~0.6us/trigger);
weights resident per phase; x streamed in 1MB token blocks; the first
wq chunk is split so the boot-critical first matmul depends on ~32KB;
the final output group is DMA'd per token-tile to shorten the drain.
"""
import os
import sys
import math
from contextlib import ExitStack

if os.path.isdir("/opt/trn_rl_repo") and "/opt/trn_rl_repo" not in sys.path:
    sys.path.insert(0, "/opt/trn_rl_repo")

import numpy as np
import ml_dtypes

import concourse.bass as bass
import concourse.tile as tile
from concourse import bacc, mybir
from concourse.bass_utils import run_bass_kernel_spmd

EMBED, HEADS, B, T = 2048, 16, 4, 2048
HD = EMBED // HEADS          # 128 head dim
NCORES = 8
HPC = HEADS // 2             # 8 heads per core
CD = HPC * HD                # 1024 local head-concat dim
SCALE = 1.0 / math.sqrt(HD)
# Wq/Wk are pre-scaled by 2**6 on the host so their ~U(-0.022, 0.022)
# entries land in fp8e4's normal range (min normal 2^-6) instead of being
# crushed to subnormals. Q'K'^T = 4096 * QK^T; fold the compensation into
# the exp scale for free.
WSCALE = 64.0
SCALE_Q = SCALE / (WSCALE * WSCALE)

F32 = mybir.dt.float32
F16 = mybir.dt.float16
FP8 = mybir.dt.float8e4
F16_NP = np.float16
F8_NP = ml_dtypes.float8_e4m3

_CACHE = {}


def _build():
    nc = bacc.Bacc("TRN2", target_bir_lowering=False, debug=False,
                   num_devices=NCORES)
    xq = nc.dram_tensor("xq", [EMBED, T], FP8, kind="ExternalInput").ap()
    xk = nc.dram_tensor("xk", [EMBED, T], FP8, kind="ExternalInput").ap()
    xv = nc.dram_tensor("xv", [EMBED, T], F16, kind="ExternalInput").ap()
    wq = nc.dram_tensor("wq", [EMBED, CD], FP8, kind="ExternalInput").ap()
    wk = nc.dram_tensor("wk", [EMBED, CD], FP8, kind="ExternalInput").ap()
    wv = nc.dram_tensor("wv", [EMBED, CD], F16, kind="ExternalInput").ap()
    wo = nc.dram_tensor("wo", [CD, EMBED], F16, kind="ExternalInput").ap()
    p = nc.dram_tensor("p", [T, EMBED], F32, kind="ExternalOutput").ap()

    ET = EMBED // 128        # 16 contraction tiles over embed
    XB = 512                 # token width of streamed x blocks
    NTB = T // XB            # 4
    DR = mybir.MatmulPerfMode.DoubleRow

    with tile.TileContext(nc) as tc, ExitStack() as ctx:
        with ExitStack() as qkv_ctx:
            qt_pool = qkv_ctx.enter_context(tc.tile_pool(name="qt", bufs=1))
            kt_pool = qkv_ctx.enter_context(tc.tile_pool(name="kt", bufs=1))
            v_pool = qkv_ctx.enter_context(tc.tile_pool(name="v", bufs=1))
            qt_sb = qt_pool.tile([128, HPC, T], F16)  # Q^T: [d, h, q]
            kt_sb = kt_pool.tile([128, HPC, T], F16)  # K^T: [d, h, k]
            v_sb = v_pool.tile([128, T // 128, CD], F16)  # V: [tok, tt, c]

            xq_r = xq.rearrange("(e p) t -> p e t", p=128)
            xk_r = xk.rearrange("(e p) t -> p e t", p=128)
            xv_r = xv.rearrange("(e p) t -> p e t", p=128)
            wq_r = wq.rearrange("(e p) c -> p e c", p=128)
            wk_r = wk.rearrange("(e p) c -> p e c", p=128)
            wv_r = wv.rearrange("(e p) c -> p e c", p=128)
            EH = ET // 2

            def proj_qk(x_r, w_sb, out_sb, load_xb):
                # one [128,512]-psum group per (tb, ds): contract embed via
                # 8 fp8 DoubleRow matmuls
                for tb in range(NTB):
                    xb = load_xb(tb)
                    for ds in range(HPC):
                        pst = ps1.tile([128, XB], F32, tag="pp", name="pst")
                        for e in range(0, ET, 2):
                            nc.tensor.matmul(
                                pst[:],
                                w_sb[:, e:e + 2, ds * 128:(ds + 1) * 128],
                                xb[:, e:e + 2, :],
                                start=(e == 0), stop=(e == ET - 2),
                                perf_mode=DR)
                        nc.vector.tensor_copy(
                            out_sb[:, ds, tb * XB:(tb + 1) * XB], pst[:])

            with ExitStack() as pv:
                # V staging pools (outlive the Q and K scopes below)
                wvpool = pv.enter_context(tc.tile_pool(name="w1", bufs=1))
                xvpool = pv.enter_context(tc.tile_pool(name="x1", bufs=2))

                # ---------------- phase 1a: Q projection (fp8 DR) -------
                with ExitStack() as p1:
                    w8pool = p1.enter_context(tc.tile_pool(name="w18q",
                                                           bufs=1))
                    x8pool = p1.enter_context(tc.tile_pool(name="x18q",
                                                           bufs=3))
                    ps1 = p1.enter_context(
                        tc.tile_pool(name="ps1", bufs=4, space="PSUM"))

                    wq_sb = w8pool.tile([128, ET, CD], FP8, tag="w8")
                    xqb = [None] * NTB

                    def load_xq(tb):
                        xb = x8pool.tile([128, ET, XB], FP8, tag="xb8",
                                         name="xb")
                        nc.sync.dma_start(
                            out=xb[:, :, :],
                            in_=xq_r[:, :, tb * XB:(tb + 1) * XB])
                        return xb

                    # q-critical loads first in fine-grained interleaved
                    # chunks, then v staging prefetch.  The first psum
                    # group only touches w[:, 0:2, 0:128] and x[:, 0:2, :];
                    # splitting those descriptors out makes the first
                    # LDWEIGHTS depend on ~32KB.
                    xqb[0] = x8pool.tile([128, ET, XB], FP8, tag="xb8",
                                         name="xb")
                    EQ = ET // 8
                    nc.scalar.dma_start(out=xqb[0][:, 0:EQ, :],
                                        in_=xq_r[:, 0:EQ, 0:XB])
                    nc.sync.dma_start(out=wq_sb[:, 0:EQ, 0:128],
                                      in_=wq_r[:, 0:EQ, 0:128])
                    nc.sync.dma_start(out=wq_sb[:, 0:EQ, 128:CD],
                                      in_=wq_r[:, 0:EQ, 128:CD])
                    for c in range(1, 8):
                        es = slice(c * EQ, (c + 1) * EQ)
                        # issue the boot-critical loads from two otherwise
                        # idle engine queues in parallel with sync's
                        # preamble
                        nc.scalar.dma_start(out=xqb[0][:, es, :],
                                            in_=xq_r[:, es, 0:XB])
                        nc.sync.dma_start(out=wq_sb[:, es, :],
                                          in_=wq_r[:, es, :])
                    xqb[1] = load_xq(1)
                    xqb[2] = load_xq(2)

                    wv_sb = wvpool.tile([128, ET, CD], F16, tag="w",
                                        name="wv_sb")
                    nc.sync.dma_start(out=wv_sb[:, 0:EH, :],
                                      in_=wv_r[:, 0:EH, :])
                    nc.sync.dma_start(out=wv_sb[:, EH:ET, :],
                                      in_=wv_r[:, EH:ET, :])

                    def load_xv(tb):
                        xb = xvpool.tile([128, ET, XB], F16, tag="xb",
                                         name="xvb")
                        nc.sync.dma_start(
                            out=xb[:, :, :],
                            in_=xv_r[:, :, tb * XB:(tb + 1) * XB])
                        return xb

                    xvb = load_xv(0)

                    def q_feed(tb):
                        if tb == NTB - 1:
                            xqb[3] = load_xq(3)
                        return xqb[tb]

                    proj_qk(xq_r, wq_sb, qt_sb, q_feed)

                # ------------- phase 1a': K staging + V projection ------
                with ExitStack() as p1:
                    w8pool = p1.enter_context(tc.tile_pool(name="w18k",
                                                           bufs=1))
                    x8pool = p1.enter_context(tc.tile_pool(name="x18k",
                                                           bufs=3))
                    ps1v = p1.enter_context(
                        tc.tile_pool(name="ps1v", bufs=4, space="PSUM"))

                    wk_sb = w8pool.tile([128, ET, CD], FP8, tag="w8")
                    xkb = [None] * NTB

                    def load_xk(tb):
                        xb = x8pool.tile([128, ET, XB], FP8, tag="xb8",
                                         name="xb")
                        nc.sync.dma_start(
                            out=xb[:, :, :],
                            in_=xk_r[:, :, tb * XB:(tb + 1) * XB])
                        return xb

                    nc.sync.dma_start(out=wk_sb[:, 0:EH, :],
                                      in_=wk_r[:, 0:EH, :])
                    xkb[0] = load_xk(0)

                    for tb in range(NTB):
                        xb = xvb if tb == 0 else load_xv(tb)
                        if tb == 1:
                            nc.sync.dma_start(out=wk_sb[:, EH:ET, :],
                                              in_=wk_r[:, EH:ET, :])
                            xkb[1] = load_xk(1)
                        for ts in range(XB // 128):
                            tt = tb * (XB // 128) + ts
                            for db in range(CD // 512):
                                pst = ps1v.tile([128, 512], F32, tag="ppv",
                                                name="pst")
                                for e in range(ET):
                                    nc.tensor.matmul(
                                        pst[:],
                                        xb[:, e, ts * 128:(ts + 1) * 128],
                                        wv_sb[:, e, db * 512:(db + 1) * 512],
                                        start=(e == 0), stop=(e == ET - 1))
                                nc.vector.tensor_copy(
                                    v_sb[:, tt, db * 512:(db + 1) * 512],
                                    pst[:])

                    # ------------- phase 1a'': K projection (fp8 DR) ----
                    ps1 = ps1v

                    def k_feed(tb):
                        if tb >= 2:
                            xkb[tb] = load_xk(tb)
                        return xkb[tb]

                    proj_qk(xk_r, wk_sb, kt_sb, k_feed)

            # o_pool created only now (lands in the staging space freed
            # above) to keep phase-1 SBUF under the per-partition cap
            o_pool = qkv_ctx.enter_context(tc.tile_pool(name="o", bufs=1))
            o_sb = o_pool.tile([128, HPC, T], F16)   # O^T: [d, h, q]

            # out-proj weights prefetch during attention (resident; the
            # phase-3 matmuls then never wait on DMA)
            wopool = qkv_ctx.enter_context(tc.tile_pool(name="wo3", bufs=1))
            wo_sb = wopool.tile([128, HPC, EMBED], F16, tag="wo")
            wo_r = wo.rearrange("(c p) e -> p c e", p=128)
            nc.sync.dma_start(out=wo_sb[:, :, 0:EMBED // 2],
                              in_=wo_r[:, :, 0:EMBED // 2])
            nc.sync.dma_start(out=wo_sb[:, :, EMBED // 2:EMBED],
                              in_=wo_r[:, :, EMBED // 2:EMBED])

            # ---------------- phase 2: attention ----------------
            with ExitStack() as p2:
                # deep e pools: exp(s) has a WAR wait on O(s-bufs) and
                # cast(s-bufs) via its pool slot; at 6 deep every PE
                # hiccup (block boundaries) propagated into the ACT
                # stream.  12 slots of lag absorb them.
                epool = p2.enter_context(tc.tile_pool(name="e2", bufs=12))
                e8pool = p2.enter_context(tc.tile_pool(name="e8", bufs=8))
                rpool = p2.enter_context(tc.tile_pool(name="r2", bufs=2))
                cpool = p2.enter_context(tc.tile_pool(name="c2", bufs=1))
                stps = p2.enter_context(
                    tc.tile_pool(name="st", bufs=2, space="PSUM"))
                otps = p2.enter_context(
                    tc.tile_pool(name="otp", bufs=2, space="PSUM"))
                smps = p2.enter_context(
                    tc.tile_pool(name="sm", bufs=2, space="PSUM"))

                # all-ones fp8 stationary [128, 2, 128]: one DoubleRow
                # matmul per exp-pair contracts 256 k-positions and lands the
                # softmax denominators replicated on every partition (full
                # width so the normalize multiply needs no cross-partition
                # broadcast)
                ones8 = cpool.tile([128, 2, 128], FP8)
                nc.vector.memset(ones8[:], 1.0)

                KT_N = T // 128   # 16 k tiles
                NP = KT_N // 2    # 8 pairs; exp runs on [128, 1024]

                def emit_st_pair(h, qb, pi):
                    # high_priority pins the S matmuls ahead of the
                    # exp-gated O matmuls of the previous slot in the
                    # scheduled PE order — otherwise the prefetched pair
                    # lands at the slot's end and the ACT stream sees it
                    # ~190ns late at every block boundary.
                    st = stps.tile([128, 1024], F32, tag="st", name="st")
                    with tc.high_priority(offset=64):
                        for j in range(2):
                            kt = 2 * pi + j
                            nc.tensor.matmul(
                                st[:, j * 512:(j + 1) * 512],
                                kt_sb[:, h, kt * 128:(kt + 1) * 128],
                                qt_sb[:, h, qb * 512:(qb + 1) * 512],
                                start=True, stop=True)
                    return st

                # Flattened (block, pair) pipeline. Slot s does:
                #   - st matmuls for slot s+1 (PE; its psum WAR is long
                #     resolved)
                #   - exp for slot s (ACT, self-paced back-to-back)
                #   - fp8 cast for slot s (DVE)
                #   - O matmuls for slot s (PE, gated only by exp s)
                #   - sm matmul for slot s-2 (PE). Two slots late because
                #     the walrus scheduler is free to hoist it ahead of the
                #     O matmuls within its slot, and its input (the DVE
                #     cast, itself gated on exp) only lands ~1.7 slots after
                #     the slot opens; at two slots' distance the PE never
                #     waits on it regardless of placement.
                blocks = [(h, qb) for h in range(HPC)
                          for qb in range(T // 512)]
                slots = [(bi, pi) for bi in range(len(blocks))
                         for pi in range(NP)]
                NS = len(slots)

                ots, sms = {}, {}
                e8s = {}

                def finish_block(b):
                    h, qb = blocks[b]
                    rbc = rpool.tile([128, 512], F32, tag="rbc",
                                     name="rbc")
                    nc.vector.reciprocal_approx_fast(rbc[:], sms[b][:])
                    nc.vector.tensor_mul(
                        o_sb[:, h, qb * 512:(qb + 1) * 512],
                        ots[b][:], rbc[:])
                    del ots[b], sms[b]

                def emit_sm(s):
                    b2, p2_ = slots[s]
                    nc.tensor.matmul(
                        sms[b2][:], ones8[:], e8s.pop(s)[:],
                        start=(p2_ == 0), stop=(p2_ == NP - 1),
                        perf_mode=DR)
                    if p2_ == NP - 1:
                        finish_block(b2)

                st_cur = emit_st_pair(*blocks[0], 0)
                st_nxt = None
                for si, (bi, pi) in enumerate(slots):
                    h, qb = blocks[bi]
                    if pi == 0:
                        ots[bi] = otps.tile([128, 512], F32, tag="ot",
                                            name="ot")
                        sms[bi] = smps.tile([128, 512], F32, tag="sm",
                                            name="sm")
                    # prefetch next slot's S pair so ACT never drains
                    if si + 1 < NS:
                        nbi, npi = slots[si + 1]
                        st_nxt = emit_st_pair(*blocks[nbi], npi)
                    e_sb = epool.tile([128, 1024], F16, tag="e",
                                      name="e_sb")
                    nc.scalar.activation(
                        e_sb[:], st_cur[:],
                        mybir.ActivationFunctionType.Exp,
                        scale=SCALE_Q)
                    e8s[si] = e8pool.tile([128, 2, 512], FP8, tag="e8",
                                          name="e8")
                    nc.vector.tensor_copy(e8s[si][:], e_sb[:])
                    for j in range(2):
                        kt = 2 * pi + j
                        nc.tensor.matmul(
                            ots[bi][:],
                            v_sb[:, kt, h * 128:(h + 1) * 128],
                            e_sb[:, j * 512:(j + 1) * 512],
                            start=(kt == 0), stop=(kt == KT_N - 1))
                    if si >= 2:
                        emit_sm(si - 2)
                    st_cur = st_nxt
                # drain the final two slots' denominators
                emit_sm(NS - 2)
                emit_sm(NS - 1)

            # ---------------- phase 3: output projection ----------------
            with ExitStack() as p3:
                ppool = p3.enter_context(tc.tile_pool(name="po3", bufs=4))
                ps3 = p3.enter_context(
                    tc.tile_pool(name="ps3", bufs=4, space="PSUM"))
                # batch output writes: 4 token tiles per DMA trigger (16
                # triggers instead of 64 on the in-order sync queue); the
                # final group goes out per-tile so the drain tail is a
                # 256KB transfer, not 1MB
                p_r = p.rearrange("(tt p) e -> p tt e", p=128)
                NEB, NTG = EMBED // 512, T // 512
                for eb in range(NEB):
                    for tg in range(NTG):
                        last = (eb == NEB - 1 and tg == NTG - 1)
                        pg = ppool.tile([128, 4, 512], F32, tag="po",
                                        name="pg")
                        for ti in range(4):
                            tt = tg * 4 + ti
                            pst = ps3.tile([128, 512], F32, tag="pp3",
                                           name="pst")
                            for ct in range(HPC):
                                nc.tensor.matmul(
                                    pst[:],
                                    o_sb[:, ct, tt * 128:(tt + 1) * 128],
                                    wo_sb[:, ct, eb * 512:(eb + 1) * 512],
                                    start=(ct == 0), stop=(ct == HPC - 1))
                            nc.scalar.copy(pg[:, ti, :], pst[:])
                            if last:
                                nc.sync.dma_start(
                                    out=p_r[:, tt:tt + 1,
                                            eb * 512:(eb + 1) * 512],
                                    in_=pg[:, ti:ti + 1, :])
                        if not last:
                            nc.sync.dma_start(
                                out=p_r[:, tg * 4:(tg + 1) * 4,
                                        eb * 512:(eb + 1) * 512],
                                in_=pg[:, :, :])

    nc.compile()
    return nc


def _get_nc():
    if "nc" not in _CACHE:
        _CACHE["nc"] = _build()
    return _CACHE["nc"]


def kernel(k, q, v, Wk, Wq, Wv, Wo, bo, _trace=False):
    k = np.asarray(k, dtype=np.float32)
    q = np.asarray(q, dtype=np.float32)
    v = np.asarray(v, dtype=np.float32)
    Wk = np.asarray(Wk, dtype=np.float32)
    Wq = np.asarray(Wq, dtype=np.float32)
    Wv = np.asarray(Wv, dtype=np.float32)
    Wo = np.asarray(Wo, dtype=np.float32)
    bo = np.asarray(bo, dtype=np.float32)

    nc = _get_nc()

    # host-side shard prep (q/k in fp8, v in fp16)
    xqT = [np.ascontiguousarray(q[b].T).astype(F8_NP) for b in range(B)]
    xkT = [np.ascontiguousarray(k[b].T).astype(F8_NP) for b in range(B)]
    xvT = [np.ascontiguousarray(v[b].T).astype(F16_NP) for b in range(B)]
    WqT = (Wq.T * WSCALE).astype(F8_NP)
    WkT = (Wk.T * WSCALE).astype(F8_NP)
    WvT = Wv.T.astype(F16_NP)
    WoT = Wo.T.astype(F16_NP)

    in_maps = []
    for c in range(NCORES):
        b, half = divmod(c, 2)
        sl = slice(half * CD, (half + 1) * CD)
        in_maps.append({
            "xq": xqT[b], "xk": xkT[b], "xv": xvT[b],
            "wq": np.ascontiguousarray(WqT[:, sl]),
            "wk": np.ascontiguousarray(WkT[:, sl]),
            "wv": np.ascontiguousarray(WvT[:, sl]),
            "wo": np.ascontiguousarray(WoT[sl, :]),
        })

    if _trace:
        try:
            res = run_bass_kernel_spmd(nc, in_maps, list(range(NCORES)),
                                       trace=True)
        except Exception as e:
            print(f"trace run failed ({e!r}); retrying without trace",
                  file=sys.stderr)
            res = run_bass_kernel_spmd(nc, in_maps, list(range(NCORES)))
    else:
        res = run_bass_kernel_spmd(nc, in_maps, list(range(NCORES)))
    _CACHE["exec_time_ns"] = res.exec_time_ns
    _CACHE["trace"] = res.instructions_and_trace

    out = np.empty((B, T, EMBED), dtype=np.float32)
    for b in range(B):
        out[b] = res.results[2 * b]["p"] + res.results[2 * b + 1]["p"] + bo
    return out
